# revision 28
# baseline (speedup 1.0000x reference)
"""Trainium2 Bass kernel for ActorGCN (GCNConv(1->128) + BN + Linear + ReLU + Softmax).

Rank-1 identity: x[n,:] = state[n]*W + b, so message passing collapses to a
scalar segment-sum per node: s1[d] = dinv[d]*(sum_{e: s->d} dinv[s]*state[s]
+ dinv[d]*state[d]), and BN stats collapse to scalar moments.

Scheme S/E (scan expansion + local_scatter, no gathers):
Edges are sharded across 8 NeuronCores by SOURCE shard.  Within a core the
~400K edges are laid out as 128 per-partition lanes with
lane = 16*(dst%8) + 8*src_half + (dst//8)%8 (src_half splits the 12544-entry
value table in two so each lane only scans half of it), slots sorted by
source.  Per-edge values u[src] are materialized without any gather:
  1. local_scatter #1 writes u[s] (f16, from a per-half broadcast table) at
     each (lane, source)-run start.
  2. An affine DVE scan state = m*state + v fills values forward through runs.
  3. local_scatter #2 permutes each lane's stream into dst-sorted order.
  4. A second affine scan (reset at segment starts) produces running segment
     sums; segment-end slots hold per-(lane,dst) sums.
  5. local_scatter #3 extracts segment ends into a [128 x 1568] accumulator.
  6. A one-hot matmul pair-sums the two src-half lanes -> [64 x 1568] f32.
A ReduceScatter(add) over the flat [100352] buffer gives each core its owned
dsts (d = 8c mod ...); BN stats use a tiny AllReduce; the BN/linear/softmax
tail collapses to per-node scalar coefficients.

The host only reorganizes integer edge structure (sort/bucket/flags); all
value arithmetic runs on device.
"""

import os
import sys

for _p in ("/opt/trn_rl_repo", "/root/.axon_site/_ro/trn_rl_repo"):
    if os.path.isdir(_p) and _p not in sys.path:
        sys.path.append(_p)

import numpy as np

# ---------------------------------------------------------------------------
N = 100000
E = 3200000
H = 128
OUT = 2
BN_EPS = 1e-5
NCORES = 8

SH = 12544               # nodes per source shard (= 128*98)
NPP = 98
NTOT = NCORES * SH       # 100352 padded node space
NLANE = 128
HALF = SH // 2           # 6272: value-table half per lane
OPL = NTOT // 64         # 1568 dst offsets per lane (dst//64)
PAD_DEG = 1.0e30

_LAST_EXEC_NS = None


# ---------------------------------------------------------------------------
def _host_prep_s(state, edge_index):
    src = np.asarray(edge_index[0], dtype=np.int64)
    dst = np.asarray(edge_index[1], dtype=np.int64)
    deg = np.bincount(dst, minlength=N).astype(np.float64) + 1.0

    state_f = np.asarray(state, dtype=np.float32)
    deg_pad = np.full(NTOT, PAD_DEG, dtype=np.float32)
    deg_pad[:N] = deg.astype(np.float32)
    state_pad = np.zeros(NTOT, dtype=np.float32)
    state_pad[:N] = state_f

    pcs = []
    for c in range(NCORES):
        lo = c * SH
        sel = (src >= lo) & (src < lo + SH)
        s = (src[sel] - lo).astype(np.int32)
        d = dst[sel].astype(np.int32)
        half = (s >= HALF).astype(np.int32)
        lane = 64 * half + 8 * (d % 8) + (d // 8) % 8
        off = d // 64
        sloc = s % HALF

        order = np.lexsort((s, lane))
        s, lane, off, sloc = s[order], lane[order], off[order], sloc[order]
        lane_cnt = np.bincount(lane, minlength=NLANE)
        lane_start = np.zeros(NLANE + 1, dtype=np.int64)
        np.cumsum(lane_cnt, out=lane_start[1:])
        slotA = np.arange(len(s), dtype=np.int64) - lane_start[lane]
        newrun = np.ones(len(s), dtype=bool)
        newrun[1:] = ~((lane[1:] == lane[:-1]) & (s[1:] == s[:-1]))

        orderB = np.lexsort((off, lane))
        laneB, offB = lane[orderB], off[orderB]
        lsB = np.zeros(NLANE + 1, dtype=np.int64)
        np.cumsum(np.bincount(laneB, minlength=NLANE), out=lsB[1:])
        slotB_B = np.arange(len(laneB), dtype=np.int64) - lsB[laneB]
        posB = np.empty(len(s), dtype=np.int64)
        posB[orderB] = slotB_B

        segstart = np.ones(len(laneB), dtype=bool)
        sameB = (laneB[1:] == laneB[:-1]) & (offB[1:] == offB[:-1])
        segstart[1:] = ~sameB
        segend = np.ones(len(laneB), dtype=bool)
        segend[:-1] = ~sameB

        pcs.append(dict(
            s=s, lane=lane, slotA=slotA, sloc=sloc, newrun=newrun, posB=posB,
            laneB=laneB, offB=offB, slotB_B=slotB_B,
            segstart=segstart, segend=segend,
            Wc=int(lane_cnt.max()),
        ))

    W = max(p["Wc"] for p in pcs)
    W = -(-W // 128) * 128
    W0 = W // 2
    assert W0 <= 2046, f"window {W0} exceeds local_scatter cap"

    b0 = 0
    a1 = HALF
    for p in pcs:
        st = p["slotA"][p["newrun"]]
        sl = p["sloc"][p["newrun"]]
        in0 = st < W0
        if in0.any():
            b0 = max(b0, int(sl[in0].max()) + 1)
        if (~in0).any():
            a1 = min(a1, int(sl[~in0].min()))
    b0 = min(HALF, -(-b0 // 2) * 2)
    a1 = (a1 // 2) * 2
    n1a, n1b = b0, HALF - a1

    in_maps = []
    for p in pcs:
        s, lane, slotA, sloc = p["s"], p["lane"], p["slotA"], p["sloc"]
        newrun, posB = p["newrun"], p["posB"]
        laneB, offB, slotB_B = p["laneB"], p["offB"], p["slotB_B"]

        i1a = np.full((NLANE, n1a), -1, dtype=np.int16)
        i1b = np.full((NLANE, n1b), -1, dtype=np.int16)
        st = slotA[newrun]
        sl = sloc[newrun]
        ln = lane[newrun]
        in0 = st < W0
        i1a[ln[in0], sl[in0]] = st[in0].astype(np.int16)
        i1b[ln[~in0], (sl[~in0] - a1)] = (st[~in0] - W0).astype(np.int16)

        mA = np.ones((NLANE, W), dtype=np.float16)
        mA[ln, st] = 0.0

        iB0 = np.full((NLANE, W), -1, dtype=np.int16)
        iB1 = np.full((NLANE, W), -1, dtype=np.int16)
        pb_lo = posB < W0
        iB0[lane[pb_lo], slotA[pb_lo]] = posB[pb_lo].astype(np.int16)
        iB1[lane[~pb_lo], slotA[~pb_lo]] = (posB[~pb_lo] - W0).astype(np.int16)

        mB = np.ones((NLANE, W), dtype=np.float16)
        ssl = p["segstart"]
        mB[laneB[ssl], slotB_B[ssl]] = 0.0

        iC = np.full((NLANE, W), -1, dtype=np.int16)
        se = p["segend"]
        iC[laneB[se], slotB_B[se]] = offB[se].astype(np.int16)

        in_maps.append(dict(i1a=i1a, i1b=i1b, mA=mA, iB0=iB0, iB1=iB1,
                            mB=mB, iC=iC))

    # pair-sum one-hot: row m sums partitions m and m+64 (the two src halves)
    pairP = np.concatenate([np.eye(64, dtype=np.float16),
                            np.eye(64, dtype=np.float16)], axis=0)

    i_ = np.arange(SH, dtype=np.int64)
    for c in range(NCORES):
        lo = c * SH
        in_maps[c]["deg_sh"] = deg_pad[lo:lo + SH].copy()
        in_maps[c]["state_sh"] = state_pad[lo:lo + SH].copy()
        # owned dsts: rs_out[i] is pair m=8c+i//OPL, o=i%OPL ->
        # dst = 64*o + 8*j + c with j = i//OPL
        dstg = 64 * (i_ % OPL) + 8 * (i_ // OPL) + c
        in_maps[c]["deg_own"] = deg_pad[dstg].copy()
        in_maps[c]["state_own"] = state_pad[dstg].copy()
        in_maps[c]["pairP"] = pairP

    return in_maps, W, b0, a1


# ---------------------------------------------------------------------------
def _build_nc_s(W, b0, a1):
    import concourse.tile as tile
    from concourse import bacc, mybir

    f32 = mybir.dt.float32
    f16 = mybir.dt.float16
    i16 = mybir.dt.int16
    AF = mybir.ActivationFunctionType
    ALU = mybir.AluOpType

    W0 = W // 2
    n1a, n1b = b0, HALF - a1

    nc = bacc.Bacc("TRN2", target_bir_lowering=False, debug=False,
                   num_devices=NCORES)

    deg_sh = nc.dram_tensor("deg_sh", [SH], f32, kind="ExternalInput").ap()
    state_sh = nc.dram_tensor("state_sh", [SH], f32, kind="ExternalInput").ap()
    deg_own = nc.dram_tensor("deg_own", [SH], f32, kind="ExternalInput").ap()
    state_own = nc.dram_tensor("state_own", [SH], f32, kind="ExternalInput").ap()
    pairP_t = nc.dram_tensor("pairP", [NLANE, 64], f16, kind="ExternalInput").ap()
    i1a_t = nc.dram_tensor("i1a", [NLANE, n1a], i16, kind="ExternalInput").ap()
    i1b_t = nc.dram_tensor("i1b", [NLANE, n1b], i16, kind="ExternalInput").ap()
    mA_t = nc.dram_tensor("mA", [NLANE, W], f16, kind="ExternalInput").ap()
    iB0_t = nc.dram_tensor("iB0", [NLANE, W], i16, kind="ExternalInput").ap()
    iB1_t = nc.dram_tensor("iB1", [NLANE, W], i16, kind="ExternalInput").ap()
    mB_t = nc.dram_tensor("mB", [NLANE, W], f16, kind="ExternalInput").ap()
    iC_t = nc.dram_tensor("iC", [NLANE, W], i16, kind="ExternalInput").ap()
    gcn_W = nc.dram_tensor("gcn_W", [1, H], f32, kind="ExternalInput").ap()
    bn_gamma = nc.dram_tensor("bn_gamma", [H], f32, kind="ExternalInput").ap()
    bn_beta = nc.dram_tensor("bn_beta", [H], f32, kind="ExternalInput").ap()
    lin_W = nc.dram_tensor("lin_W", [H, OUT], f32, kind="ExternalInput").ap()
    lin_b = nc.dram_tensor("lin_b", [OUT], f32, kind="ExternalInput").ap()
    out_t = nc.dram_tensor("out", [SH, OUT], f32, kind="ExternalOutput").ap()

    u_stage = nc.dram_tensor("u_stage", [SH], f16)
    rs_in = nc.dram_tensor("rs_in", [NTOT], f32)
    rs_out = nc.dram_tensor("rs_out", [SH], f32)
    ar_in = nc.dram_tensor("ar_in", [8], f32)
    ar_out = nc.dram_tensor("ar_out", [8], f32, addr_space="Shared")
    coef_stage = nc.dram_tensor("coef_stage", [OUT, 2], f32)

    replica = [list(range(NCORES))]

    from contextlib import ExitStack

    with tile.TileContext(nc) as tc, ExitStack() as ctx:
        pp = ctx.enter_context(tc.tile_pool(name="pp", bufs=1))
        small = ctx.enter_context(tc.tile_pool(name="sm", bufs=2))
        psum = ctx.enter_context(tc.tile_pool(name="ps", bufs=1, space="PSUM"))

        # ---- u table (natural (p n) layout) ------------------------------
        t_deg = pp.tile([128, NPP], f32)
        nc.sync.dma_start(t_deg[:], deg_sh.rearrange("(p n) -> p n", p=128))
        t_state = pp.tile([128, NPP], f32)
        nc.sync.dma_start(t_state[:], state_sh.rearrange("(p n) -> p n", p=128))

        # structure needed first by lsc#1 — issue right after the tiny tables
        t_i1a = pp.tile([128, n1a], i16)
        nc.sync.dma_start(t_i1a[:], i1a_t[:])
        t_i1b = pp.tile([128, n1b], i16)
        nc.sync.dma_start(t_i1b[:], i1b_t[:])

        t_tmp = pp.tile([128, NPP], f32)
        t_dinv = pp.tile([128, NPP], f32)
        nc.vector.reciprocal(t_tmp[:], t_deg[:])
        nc.scalar.activation(t_dinv[:], t_tmp[:], AF.Sqrt)
        t_u32 = pp.tile([128, NPP], f32)
        nc.vector.tensor_mul(t_u32[:], t_dinv[:], t_state[:])
        t_u16 = pp.tile([128, NPP], f16)
        nc.vector.tensor_copy(t_u16[:], t_u32[:])
        # u staging + broadcasts on the Scalar engine's DMA queue so the big
        # structure loads on the sync queue are not head-of-line blocked
        nc.scalar.dma_start(u_stage.ap().rearrange("(p n) -> p n", p=128),
                            t_u16[:])

        # per-half table broadcasts: partitions [0,64) hold half 0, [64,128) half 1
        t_utab_a = pp.tile([128, n1a], f16)
        t_utab_b = pp.tile([128, n1b], f16)
        for h in range(2):
            nc.scalar.dma_start(
                t_utab_a[64 * h: 64 * h + 64, :],
                u_stage.ap()[h * HALF: h * HALF + b0].partition_broadcast(64))
            nc.scalar.dma_start(
                t_utab_b[64 * h: 64 * h + 64, :],
                u_stage.ap()[h * HALF + a1: (h + 1) * HALF].partition_broadcast(64))

        # small tail inputs — issue early, they are tiny and off the queue fast
        t_P = pp.tile([128, 64], f16)
        nc.sync.dma_start(t_P[:], pairP_t[:])
        t_W = small.tile([128, 1], f32)
        nc.sync.dma_start(t_W[:], gcn_W.rearrange("o h -> h o"))
        t_gam = small.tile([128, 1], f32)
        nc.sync.dma_start(t_gam[:], bn_gamma.rearrange("(h o) -> h o", o=1))
        t_bet = small.tile([128, 1], f32)
        nc.sync.dma_start(t_bet[:], bn_beta.rearrange("(h o) -> h o", o=1))
        t_lW = small.tile([128, OUT], f32)
        nc.sync.dma_start(t_lW[:], lin_W[:])
        t_lb = small.tile([OUT, 1], f32)
        nc.sync.dma_start(t_lb[:], lin_b.rearrange("(o k) -> o k", k=1))

        t_mA = pp.tile([128, W], f16)
        nc.sync.dma_start(t_mA[:], mA_t[:])
        t_iB0 = pp.tile([128, W], i16)
        nc.sync.dma_start(t_iB0[:], iB0_t[:])
        t_iB1 = pp.tile([128, W], i16)
        nc.sync.dma_start(t_iB1[:], iB1_t[:])
        t_mB = pp.tile([128, W], f16)
        nc.sync.dma_start(t_mB[:], mB_t[:])
        t_iC = pp.tile([128, W], i16)
        nc.sync.dma_start(t_iC[:], iC_t[:])

        # ---- 1. scatter run-start values --------------------------------
        t_v0 = pp.tile([128, W], f16)
        nc.gpsimd.local_scatter(
            t_v0[:, 0:W0], t_utab_a[:], t_i1a[:],
            channels=128, num_elems=W0, num_idxs=n1a)
        nc.gpsimd.local_scatter(
            t_v0[:, W0:W], t_utab_b[:], t_i1b[:],
            channels=128, num_elems=W - W0, num_idxs=n1b)

        # ---- 2. fill-forward scan (split so half 0 overlaps lsc#1 call1) -
        t_w16 = pp.tile([128, W], f16)
        nc.vector.tensor_tensor_scan(
            t_w16[:, 0:W0], t_mA[:, 0:W0], t_v0[:, 0:W0], 0.0,
            op0=ALU.mult, op1=ALU.add)
        nc.vector.tensor_tensor_scan(
            t_w16[:, W0:W], t_mA[:, W0:W], t_v0[:, W0:W],
            t_w16[:, W0 - 1:W0], op0=ALU.mult, op1=ALU.add)

        # ---- 3. permute to dst-sorted order -----------------------------
        t_z = pp.tile([128, W], f16)
        nc.gpsimd.local_scatter(
            t_z[:, 0:W0], t_w16[:], t_iB0[:],
            channels=128, num_elems=W0, num_idxs=W)
        nc.gpsimd.local_scatter(
            t_z[:, W0:W], t_w16[:], t_iB1[:],
            channels=128, num_elems=W - W0, num_idxs=W)

        # ---- 4. segment-sum scan (split like scan A) --------------------
        t_seg = pp.tile([128, W], f16)
        nc.vector.tensor_tensor_scan(
            t_seg[:, 0:W0], t_mB[:, 0:W0], t_z[:, 0:W0], 0.0,
            op0=ALU.mult, op1=ALU.add)
        nc.vector.tensor_tensor_scan(
            t_seg[:, W0:W], t_mB[:, W0:W], t_z[:, W0:W],
            t_seg[:, W0 - 1:W0], op0=ALU.mult, op1=ALU.add)

        # ---- 5. extract segment ends ------------------------------------
        t_acc16 = pp.tile([128, OPL], f16)
        nc.gpsimd.local_scatter(
            t_acc16[:], t_seg[:], t_iC[:],
            channels=128, num_elems=OPL, num_idxs=W)

        # ---- 5b. pair-sum the two src-half lanes via one-hot matmul -----
        rs_in_v = rs_in.ap().rearrange("(p o) -> p o", p=64)
        NB = 4
        CB = OPL // NB
        ps_r = psum.tile([64, NB, 512], f32, space="PSUM")
        for j in range(NB):
            nc.tensor.matmul(ps_r[:, j, 0:CB], lhsT=t_P[:],
                             rhs=t_acc16[:, j * CB:(j + 1) * CB],
                             start=True, stop=True)
        t_red = pp.tile([64, OPL], f32)
        nc.vector.tensor_copy(
            t_red[:].rearrange("p (b c) -> p b c", b=NB), ps_r[:, :, 0:CB])
        nc.sync.dma_start(rs_in_v[:], t_red[:])

        # tail tables issued here so their DMAs overlap the edge phase
        t_deg2 = pp.tile([128, NPP], f32)
        nc.sync.dma_start(t_deg2[:], deg_own.rearrange("(p n) -> p n", p=128))
        t_state2 = pp.tile([128, NPP], f32)
        nc.sync.dma_start(t_state2[:], state_own.rearrange("(p n) -> p n", p=128))
        t_dinv2 = pp.tile([128, NPP], f32)
        t_tmp2 = pp.tile([128, NPP], f32)
        nc.vector.reciprocal(t_tmp2[:], t_deg2[:])
        nc.scalar.activation(t_dinv2[:], t_tmp2[:], AF.Sqrt)
        t_uown = pp.tile([128, NPP], f32)
        nc.vector.tensor_mul(t_uown[:], t_dinv2[:], t_state2[:])

        # ---- 6. ReduceScatter -------------------------------------------
        nc.gpsimd.collective_compute(
            "ReduceScatter", mybir.AluOpType.add,
            ins=[rs_in.ap()[:]], outs=[rs_out.ap()[:]],
            replica_groups=replica,
        )

        # ---- 7. tail -----------------------------------------------------
        t_agg = pp.tile([128, NPP], f32)
        nc.sync.dma_start(t_agg[:], rs_out.ap().rearrange("(p n) -> p n", p=128))
        t_s1 = pp.tile([128, NPP], f32)
        nc.vector.tensor_add(t_s1[:], t_agg[:], t_uown[:])
        nc.vector.tensor_mul(t_s1[:], t_s1[:], t_dinv2[:])

        NSTAT = 2
        t_pr = small.tile([128, NSTAT], f32)
        t_sq = small.tile([128, NPP], f32)
        nc.vector.tensor_reduce(t_pr[:, 0:1], t_s1[:], axis=mybir.AxisListType.X,
                                op=ALU.add)
        nc.vector.tensor_mul(t_sq[:], t_s1[:], t_s1[:])
        nc.vector.tensor_reduce(t_pr[:, 1:2], t_sq[:], axis=mybir.AxisListType.X,
                                op=ALU.add)
        t_ones = small.tile([128, 1], f32)
        nc.vector.memset(t_ones[:], 1.0)
        ps_st = psum.tile([NSTAT, 1], f32, space="PSUM")
        nc.tensor.matmul(ps_st[:], lhsT=t_pr[:], rhs=t_ones[:], start=True,
                         stop=True)
        t_st = small.tile([NSTAT, 1], f32)
        nc.vector.tensor_copy(t_st[:], ps_st[:])
        nc.sync.dma_start(ar_in.ap()[0:NSTAT], t_st[:].rearrange("p n -> (p n)"))
        t_z8 = small.tile([1, 8 - NSTAT], f32)
        nc.vector.memset(t_z8[:], 0.0)
        nc.sync.dma_start(ar_in.ap()[NSTAT:8], t_z8[:].rearrange("p n -> (p n)"))

        # stats-independent pieces (overlap the AllReduce / edge phase)
        t_w2 = small.tile([128, 1], f32)
        nc.vector.tensor_mul(t_w2[:], t_W[:], t_W[:])
        t_kA = small.tile([128, OUT], f32)  # gamma*W*linW[:,o]
        t_gw = small.tile([128, 1], f32)
        nc.vector.tensor_mul(t_gw[:], t_gam[:], t_W[:])
        for o in range(OUT):
            nc.vector.tensor_mul(t_kA[:, o:o + 1], t_gw[:], t_lW[:, o:o + 1])
        ps_b = psum.tile([OUT, 1], f32, space="PSUM")
        nc.tensor.matmul(ps_b[:], lhsT=t_lW[:], rhs=t_bet[:], start=True,
                         stop=True)
        t_beto = small.tile([OUT, 1], f32)
        nc.vector.tensor_copy(t_beto[:], ps_b[:])
        nc.vector.tensor_add(t_beto[:], t_beto[:], t_lb[:])  # bet_o + lin_b

        nc.gpsimd.collective_compute(
            "AllReduce", mybir.AluOpType.add,
            ins=[ar_in.ap()[:]], outs=[ar_out.ap()[:]],
            replica_groups=replica,
        )
        t_stats = small.tile([128, 8], f32)
        nc.sync.dma_start(t_stats[:], ar_out.ap().partition_broadcast(128))

        inv_n = 1.0 / float(N)
        t_m = small.tile([128, 2], f32)
        nc.vector.tensor_scalar_mul(t_m[:, 0:1], t_stats[:, 0:1], inv_n)
        nc.vector.tensor_scalar_mul(t_m[:, 1:2], t_stats[:, 1:2], inv_n)
        t_t1 = small.tile([128, 1], f32)
        nc.vector.tensor_mul(t_t1[:], t_m[:, 0:1], t_m[:, 0:1])
        nc.vector.tensor_tensor(t_m[:, 1:2], t_m[:, 1:2], t_t1[:],
                                op=ALU.subtract)

        # isd = rsqrt(c11*W^2 + eps); Ao = kA*isd
        t_vpe = small.tile([128, 1], f32)
        nc.vector.tensor_scalar(t_vpe[:], t_w2[:], t_m[:, 1:2], BN_EPS,
                                op0=ALU.mult, op1=ALU.add)
        t_isd = small.tile([128, 1], f32)
        nc.vector.reciprocal(t_vpe[:], t_vpe[:])
        nc.scalar.activation(t_isd[:], t_vpe[:], AF.Sqrt)
        t_Ao = small.tile([128, OUT], f32)
        nc.vector.tensor_scalar_mul(t_Ao[:], t_kA[:], t_isd[:, 0:1])

        ps_c = psum.tile([OUT, 1], f32, space="PSUM")
        nc.tensor.matmul(ps_c[:], lhsT=t_Ao[:], rhs=t_ones[:], start=True,
                         stop=True)
        t_co = small.tile([OUT, 1], f32)
        nc.vector.tensor_copy(t_co[:], ps_c[:])

        t_cfin = small.tile([OUT, 2], f32)  # per-o: [a, c]
        nc.vector.tensor_copy(t_cfin[:, 0:1], t_co[:])
        t_ct = small.tile([OUT, 1], f32)
        nc.vector.tensor_mul(t_ct[:], t_co[:], t_m[0:OUT, 0:1])
        nc.vector.tensor_tensor(t_cfin[:, 1:2], t_beto[:], t_ct[:],
                                op=ALU.subtract)

        nc.sync.dma_start(coef_stage.ap()[:], t_cfin[:])
        t_coef = small.tile([128, OUT, 2], f32)  # [p, o, kind]
        nc.sync.dma_start(
            t_coef[:].rearrange("p o k -> p (o k)"),
            coef_stage.ap().rearrange("o k -> (o k)").partition_broadcast(128))

        t_l = pp.tile([128, NPP, OUT], f32)
        a_ap = t_coef[:, :, 0:1].rearrange("p o k -> p k o").to_broadcast(
            [128, NPP, OUT])
        c_ap = t_coef[:, :, 1:2].rearrange("p o k -> p k o").to_broadcast(
            [128, NPP, OUT])
        nc.vector.tensor_tensor(
            t_l[:], t_s1[:].rearrange("p (n k) -> p n k", k=1).to_broadcast(
                [128, NPP, OUT]),
            a_ap, op=ALU.mult)
        nc.vector.tensor_tensor(t_l[:], t_l[:], c_ap, op=ALU.add)
        nc.vector.tensor_scalar_max(
            t_l[:].rearrange("p n o -> p (n o)"),
            t_l[:].rearrange("p n o -> p (n o)"), 0.0)

        t_zd = small.tile([128, NPP], f32)
        nc.vector.tensor_tensor(t_zd[:], t_l[:, :, 1], t_l[:, :, 0],
                                op=ALU.subtract)
        t_res = pp.tile([128, NPP, OUT], f32)
        nc.scalar.activation(t_res[:, :, 1], t_zd[:], AF.Sigmoid)
        nc.vector.tensor_scalar(t_res[:, :, 0], t_res[:, :, 1], 1.0, None,
                                op0=ALU.subtract)
        nc.vector.tensor_scalar_mul(t_res[:, :, 0], t_res[:, :, 0], -1.0)

        nc.sync.dma_start(out_t.rearrange("(p n) d -> p n d", p=128), t_res[:])

    nc.compile()
    return nc


_NC_CACHE = {}


def _kernel_s(state, edge_index, gcn_W, gcn_b, bn_gamma, bn_beta, lin_W, lin_b):
    global _LAST_EXEC_NS
    from concourse.bass_utils import run_bass_kernel_spmd

    in_maps, W, b0, a1 = _host_prep_s(state, edge_index)
    key = ("s", W, b0, a1)
    if key not in _NC_CACHE:
        _NC_CACHE[key] = _build_nc_s(W, b0, a1)
    nc = _NC_CACHE[key]

    shared = {
        "gcn_W": np.asarray(gcn_W, dtype=np.float32),
        "bn_gamma": np.asarray(bn_gamma, dtype=np.float32),
        "bn_beta": np.asarray(bn_beta, dtype=np.float32),
        "lin_W": np.asarray(lin_W, dtype=np.float32),
        "lin_b": np.asarray(lin_b, dtype=np.float32),
    }
    for m in in_maps:
        m.update(shared)

    trace = os.environ.get("BASS_GCN_TRACE", "0") == "1"
    res = run_bass_kernel_spmd(nc, in_maps, list(range(NCORES)), trace=trace)
    _LAST_EXEC_NS = res.exec_time_ns

    out = np.empty((N, OUT), dtype=np.float32)
    i_ = np.arange(SH, dtype=np.int64)
    for c in range(NCORES):
        dstg = 64 * (i_ % OPL) + 8 * (i_ // OPL) + c
        valid = dstg < N
        out[dstg[valid]] = res.results[c]["out"][valid]
    return out


def kernel(state, edge_index, gcn_W, gcn_b, bn_gamma, bn_beta, lin_W, lin_b):
    global _LAST_EXEC_NS
    if float(np.abs(np.asarray(gcn_b)).max()) == 0.0:
        return _kernel_s(state, edge_index, gcn_W, gcn_b, bn_gamma, bn_beta,
                         lin_W, lin_b)
    import kernel_v1_backup as _v1
    out = _v1.kernel(state, edge_index, gcn_W, gcn_b, bn_gamma, bn_beta,
                     lin_W, lin_b)
    _LAST_EXEC_NS = _v1._LAST_EXEC_NS
    return out
